# revision 18
# baseline (speedup 1.0000x reference)
"""Griffin block (Hawk RG-LRU + GatedMLP) Trainium2 Bass kernel.

Sharding: 8 chunks = 4 batches x 2 time-halves, one per NeuronCore.
Per-core layout is feature-major ([channels, tokens]).

Key structure (v2):
  - rmsnorm commutes through the projections: (x*s) @ W == s * (x @ W)
    for the per-token scale s = 1/||x||, so every matmul runs on the RAW
    bf16 activations and s is folded into the PSUM->SBUF drain op. This
    removes the norm barrier in front of each matmul block.
  - phase 1: input proj (z+gate halves), causal dw-conv, gates proj,
    alpha/beta, RG-LRU local scan + cumulative-alpha scan. h/ac/gate are
    spilled to DRAM in bf16.
  - pairwise AllGather exchanges the half-boundary scan state.
  - phase 2: carry fixup, gelu(gate)*h, output proj (+residual), gmlp
    grow/gelu/shrink (+residual), with the x1-norm applied at drains.
  - ScalarE work is batched by activation function to minimize
    ACT_TABLE_LOADs; conv taps / casts / ac-scans ride GpSimd.
  - wo is prefetched during phase 1; wgr/wsh load at phase-2 start
    under cover of the output-proj tile stream.
"""

import numpy as np
import ml_dtypes
from contextlib import ExitStack

import concourse.bass as bass
import concourse.bacc as bacc
import concourse.tile as tile
from concourse import mybir
from concourse.bass_utils import run_bass_kernel_spmd

F32 = mybir.dt.float32
BF16 = mybir.dt.bfloat16
AF = mybir.ActivationFunctionType
OP = mybir.AluOpType

D = 1024
NP = 128          # partitions
NCT = D // NP     # channel tiles = 8
KCONV = 4
N_CORES = 8

_BF = ml_dtypes.bfloat16


def build_program(T_core: int, L: int, gelu_approx: bool = False,
                  L2: int | None = None):
    """Emit the SPMD program. T_core tokens per core, token tile L."""
    assert T_core % L == 0
    n_tiles = T_core // L
    if L2 is None:
        L2 = L
    n_tiles2 = T_core // L2
    H2 = 2 * D        # hawk proj width (2048)
    HID = 2 * H2      # gmlp hidden rows (4096): gate2 [0:2048), v [2048:4096)
    GELU = AF.Gelu_apprx_sigmoid if gelu_approx else AF.Gelu

    nc = bacc.Bacc("TRN2", target_bir_lowering=False, debug=False,
                   num_devices=N_CORES)

    # ---- DRAM parameters (per-core data via in_maps) ----
    x_d = nc.dram_tensor("x", [D, 3 + T_core], F32, kind="ExternalInput")
    wi_d = nc.dram_tensor("wi", [D, H2], BF16, kind="ExternalInput")      # input_w.T (gamma folded)
    wg_d = nc.dram_tensor("wg", [D, H2], BF16, kind="ExternalInput")      # gates_w.T
    wo_d = nc.dram_tensor("wo", [D, D], BF16, kind="ExternalInput")       # output_w.T
    wgr_d = nc.dram_tensor("wgr", [D, HID], BF16, kind="ExternalInput")   # grow_w.T (gamma folded)
    wsh_d = nc.dram_tensor("wsh", [H2, D], BF16, kind="ExternalInput")    # shrink_w.T
    # per-channel params, laid out [partition, ch_tile]
    msp_d = nc.dram_tensor("msp", [NP, NCT], F32, kind="ExternalInput")    # -8*softplus(fb)
    msp2_d = nc.dram_tensor("msp2", [NP, NCT], F32, kind="ExternalInput")  # 2*msp
    gbf_d = nc.dram_tensor("gbf", [NP, NCT], F32, kind="ExternalInput")    # gates_b[:D]
    gbi_d = nc.dram_tensor("gbi", [NP, NCT], F32, kind="ExternalInput")    # gates_b[D:]
    cw_d = nc.dram_tensor("cw", [NP, KCONV * NCT], F32, kind="ExternalInput")  # conv w taps
    cb_d = nc.dram_tensor("cb", [NP, NCT], F32, kind="ExternalInput")      # conv bias
    cmask_d = nc.dram_tensor("cmask", [NP, 1], F32, kind="ExternalInput")  # 1.0 iff second half

    out_d = nc.dram_tensor("out", [D, T_core], F32, kind="ExternalOutput")

    # ---- internal DRAM scratch ----
    h_d = nc.dram_tensor("h_spill", [D, T_core], BF16)
    ac_d = nc.dram_tensor("ac_spill", [D, T_core], BF16)
    g_d = nc.dram_tensor("g_spill", [D, T_core], BF16)
    xb_d = nc.dram_tensor("xb_spill", [D, T_core], BF16)
    carry_loc = nc.dram_tensor("carry_loc", [1, D], BF16)
    carry_gth = nc.dram_tensor("carry_gth", [2, D], BF16)

    with tile.TileContext(nc) as tc, ExitStack() as top:
        # ------- persistent small constants -------
        cpool = top.enter_context(tc.tile_pool(name="consts", bufs=1))
        ones_bf = cpool.tile([NP, NP], BF16, name="ones_bf")
        nc.vector.memset(ones_bf[:], 1.0)
        ones_f = cpool.tile([NP, L], BF16, name="ones_f")
        nc.vector.memset(ones_f[:], 1.0)
        msp_sb = cpool.tile([NP, NCT], F32, name="msp_sb")
        nc.sync.dma_start(msp_sb[:], msp_d.ap()[:, :])
        msp2_sb = cpool.tile([NP, NCT], F32, name="msp2_sb")
        nc.sync.dma_start(msp2_sb[:], msp2_d.ap()[:, :])
        gbf_sb = cpool.tile([NP, NCT], F32, name="gbf_sb")
        nc.sync.dma_start(gbf_sb[:], gbf_d.ap()[:, :])
        gbi_sb = cpool.tile([NP, NCT], F32, name="gbi_sb")
        nc.sync.dma_start(gbi_sb[:], gbi_d.ap()[:, :])
        cw_sb = cpool.tile([NP, KCONV * NCT], F32, name="cw_sb")
        nc.sync.dma_start(cw_sb[:], cw_d.ap()[:, :])
        cb_sb = cpool.tile([NP, NCT], F32, name="cb_sb")
        nc.sync.dma_start(cb_sb[:], cb_d.ap()[:, :])
        cmask_sb = cpool.tile([NP, 1], F32, name="cmask_sb")
        nc.sync.dma_start(cmask_sb[:], cmask_d.ap()[:, :])
        epsb = cpool.tile([NP, 1], F32, name="epsb")
        nc.vector.memset(epsb[:], 1e-20)
        onepb = cpool.tile([NP, 1], F32, name="onepb")
        nc.vector.memset(onepb[:], 1.0 + 1e-6)

        # phase-2 weight pool that must coexist with phase-1 pools (wo is
        # prefetched during phase 1)
        w2a = top.enter_context(tc.tile_pool(name="w2a", bufs=1))
        wo_sb = w2a.tile([NP, NCT * D], BF16, name="wo_sb")

        # =========================== PHASE 1 ===========================
        with ExitStack() as p1:
            wpool = p1.enter_context(tc.tile_pool(name="w1", bufs=1))
            wi_sb = wpool.tile([NP, NCT * H2], BF16, name="wi_sb")
            wg_sb = wpool.tile([NP, NCT * H2], BF16, name="wg_sb")
            # z-half of wi first so the z-chain can start earliest
            for k in range(NCT):
                nc.sync.dma_start(wi_sb[:, k * H2 + D:(k + 1) * H2],
                                  wi_d.ap()[k * NP:(k + 1) * NP, D:])
            for k in range(NCT):
                nc.scalar.dma_start(wg_sb[:, k * H2:(k + 1) * H2],
                                    wg_d.ap()[k * NP:(k + 1) * NP, :])
            for k in range(NCT):
                nc.sync.dma_start(wi_sb[:, k * H2:k * H2 + D],
                                  wi_d.ap()[k * NP:(k + 1) * NP, :D])

            xp = p1.enter_context(tc.tile_pool(name="xp", bufs=4))
            xbp = p1.enter_context(tc.tile_pool(name="xbp", bufs=8))
            sqp = p1.enter_context(tc.tile_pool(name="sqp", bufs=2))
            sp = p1.enter_context(tc.tile_pool(name="sp", bufs=3))
            zp = p1.enter_context(tc.tile_pool(name="zp", bufs=9))
            zcp = p1.enter_context(tc.tile_pool(name="zcp", bufs=3))
            zcbp = p1.enter_context(tc.tile_pool(name="zcbp", bufs=8))
            sfp = p1.enter_context(tc.tile_pool(name="sfp", bufs=9))
            sip = p1.enter_context(tc.tile_pool(name="sip", bufs=4))
            ap_ = p1.enter_context(tc.tile_pool(name="ap", bufs=5))
            a2p = p1.enter_context(tc.tile_pool(name="a2p", bufs=3))
            bp = p1.enter_context(tc.tile_pool(name="bp", bufs=3))
            szp = p1.enter_context(tc.tile_pool(name="szp", bufs=3))
            xsp = p1.enter_context(tc.tile_pool(name="xsp", bufs=3))
            hp = p1.enter_context(tc.tile_pool(name="hp", bufs=9))
            acp = p1.enter_context(tc.tile_pool(name="acp", bufs=9))
            gp = p1.enter_context(tc.tile_pool(name="gp", bufs=7))
            zhp = p1.enter_context(tc.tile_pool(name="zhp", bufs=1))
            pmm = p1.enter_context(
                tc.tile_pool(name="pmm", bufs=5, space="PSUM"))
            pssq = p1.enter_context(
                tc.tile_pool(name="pssq", bufs=2, space="PSUM"))

            # ---- halo: z for the 3 tokens before this chunk ----
            xh = xp.tile([NP, 3 * NCT], F32, name="xh", tag="xh")
            for i in range(NCT):
                nc.gpsimd.dma_start(xh[:, 3 * i:3 * i + 3],
                                  x_d.ap()[i * NP:(i + 1) * NP, 0:3])
            xbh = xbp.tile([NP, 3 * NCT], BF16, name="xbh", tag="xbh")
            nc.gpsimd.tensor_copy(xbh[:], xh[:])
            xsqh = sqp.tile([NP, 3 * NCT], BF16, name="xsqh", tag="xsqh")
            nc.vector.tensor_tensor(xsqh[:], xbh[:], xbh[:], OP.mult)
            ssqh = pssq.tile([NP, 3], F32, name="ssqh", tag="ssq")
            for i in range(NCT):
                nc.tensor.matmul(ssqh[:], ones_bf[:], xsqh[:, 3 * i:3 * i + 3],
                                 start=(i == 0), stop=(i == NCT - 1))
            lssqh = sp.tile([NP, 3], F32, name="lssqh", tag="lssq")
            nc.scalar.activation(lssqh[:], ssqh[:], AF.Ln, bias=epsb[:, 0:1])
            sh_ = sp.tile([NP, 3], F32, name="sh_", tag="s")
            nc.scalar.activation(sh_[:], lssqh[:], AF.Exp, scale=-0.5)
            zhalo = zhp.tile([NP, 3 * NCT], BF16, name="zhalo", tag="zhalo")
            for m in range(NCT):  # z half rows of input_w = cols [D + 128m ...)
                ps = pmm.tile([NP, 3], F32, name=f"zh_ps_{m}", tag="mm")
                for k in range(NCT):
                    lhs = wi_sb[:, k * H2 + D + m * NP: k * H2 + D + (m + 1) * NP]
                    nc.tensor.matmul(ps[:], lhs, xbh[:, 3 * k:3 * k + 3],
                                     start=(k == 0), stop=(k == NCT - 1))
                nc.vector.tensor_tensor(zhalo[:, 3 * m:3 * m + 3], ps[:],
                                        sh_[:], OP.mult)

            # ---- main phase-1 tiles ----
            h_prev = [None] * NCT
            ac_prev = [None] * NCT
            z_prev = [None] * NCT
            st_rr = [nc.scalar, nc.sync, nc.gpsimd]
            for t in range(n_tiles):
                c0 = 3 + t * L
                if t == n_tiles - 2:
                    # prefetch the output-proj weights for phase 2
                    for k in range(NCT):
                        nc.sync.dma_start(wo_sb[:, k * D:(k + 1) * D],
                                          wo_d.ap()[k * NP:(k + 1) * NP, :])
                x_t = [None] * NCT
                xb = [None] * NCT
                for i in range(NCT):
                    xi = xp.tile([NP, L], F32, name=f"x_{t}_{i}", tag="x")
                    nc.gpsimd.dma_start(
                        xi[:], x_d.ap()[i * NP:(i + 1) * NP, c0:c0 + L])
                    x_t[i] = xi
                    xbi = xbp.tile([NP, L], BF16, name=f"xb_{t}_{i}", tag="xb")
                    nc.gpsimd.tensor_copy(xbi[:], xi[:])
                    st_rr[i % 3].dma_start(
                        xb_d.ap()[i * NP:(i + 1) * NP, t * L:(t + 1) * L],
                        xbi[:])
                    xb[i] = xbi
                # token norm scale s = 1/||x|| (broadcast over partitions)
                ssq = pssq.tile([NP, L], F32, name=f"ssq_{t}", tag="ssq")
                for i in range(NCT):
                    xsq = sqp.tile([NP, L], BF16, name=f"xsq_{t}_{i}", tag="xsq")
                    nc.scalar.activation(xsq[:], x_t[i][:], AF.Square)
                    nc.tensor.matmul(ssq[:], ones_bf[:], xsq[:],
                                     start=(i == 0), stop=(i == NCT - 1))
                lssq = sp.tile([NP, L], F32, name=f"lssq_{t}", tag="lssq")
                nc.scalar.activation(lssq[:], ssq[:], AF.Ln, bias=epsb[:, 0:1])
                s = sp.tile([NP, L], F32, name=f"s_{t}", tag="s")
                nc.scalar.activation(s[:], lssq[:], AF.Exp, scale=-0.5)

                # input proj, z half first (conv chain starts early)
                z_cur = [None] * NCT
                for m in range(NCT, 2 * NCT):
                    ps = pmm.tile([NP, L], F32, name=f"u_ps_{t}_{m}", tag="mm")
                    for k in range(NCT):
                        lhs = wi_sb[:, k * H2 + m * NP: k * H2 + (m + 1) * NP]
                        nc.tensor.matmul(ps[:], lhs, xb[k][:],
                                         start=(k == 0), stop=(k == NCT - 1))
                    i = m - NCT
                    zt = zp.tile([NP, L + 3], BF16, name=f"z_{t}_{i}", tag="z")
                    nc.vector.tensor_tensor(zt[:, 3:3 + L], ps[:], s[:], OP.mult)
                    if t == 0:
                        nc.gpsimd.tensor_copy(zt[:, 0:3],
                                              zhalo[:, 3 * i:3 * i + 3])
                    else:
                        nc.gpsimd.tensor_copy(zt[:, 0:3],
                                              z_prev[i][:, L:L + 3])
                    z_cur[i] = zt
                z_prev = z_cur

                # depthwise causal conv: 4 taps, f32 accum, bf16 out
                zcb = [None] * NCT
                for i in range(NCT):
                    zci = zcp.tile([NP, L], BF16, name=f"zc_{t}_{i}", tag="zc")
                    nc.vector.tensor_scalar(
                        zci[:], z_cur[i][:, 0:L],
                        cw_sb[:, 0 * NCT + i:0 * NCT + i + 1],
                        cb_sb[:, i:i + 1], op0=OP.mult, op1=OP.add)
                    nc.vector.scalar_tensor_tensor(
                        zci[:], z_cur[i][:, 1:1 + L],
                        cw_sb[:, 1 * NCT + i:1 * NCT + i + 1],
                        zci[:], op0=OP.mult, op1=OP.add)
                    nc.vector.scalar_tensor_tensor(
                        zci[:], z_cur[i][:, 2:2 + L],
                        cw_sb[:, 2 * NCT + i:2 * NCT + i + 1],
                        zci[:], op0=OP.mult, op1=OP.add)
                    zcbi = zcbp.tile([NP, L], BF16, name=f"zcb_{t}_{i}", tag="zcb")
                    nc.vector.scalar_tensor_tensor(
                        zcbi[:], z_cur[i][:, 3:3 + L],
                        cw_sb[:, 3 * NCT + i:3 * NCT + i + 1],
                        zci[:], op0=OP.mult, op1=OP.add)
                    zcb[i] = zcbi

                # gates proj -> sigmoids (batched); then exp/sqrt chain
                sf = [None] * NCT
                si = [None] * NCT
                for i in range(NCT):
                    psf = pmm.tile([NP, L], F32, name=f"f_ps_{t}_{i}", tag="mm")
                    for k in range(NCT):
                        lhs = wg_sb[:, k * H2 + i * NP: k * H2 + (i + 1) * NP]
                        nc.tensor.matmul(psf[:], lhs, zcb[k][:],
                                         start=(k == 0), stop=(k == NCT - 1))
                    sfi = sfp.tile([NP, L], BF16, name=f"sf_{t}_{i}", tag="sf")
                    nc.scalar.activation(sfi[:], psf[:], AF.Sigmoid,
                                         bias=gbf_sb[:, i:i + 1])
                    sf[i] = sfi
                    psi = pmm.tile([NP, L], F32, name=f"i_ps_{t}_{i}", tag="mm")
                    for k in range(NCT):
                        lhs = wg_sb[:, k * H2 + D + i * NP: k * H2 + D + (i + 1) * NP]
                        nc.tensor.matmul(psi[:], lhs, zcb[k][:],
                                         start=(k == 0), stop=(k == NCT - 1))
                    sii = sip.tile([NP, L], BF16, name=f"si_{t}_{i}", tag="si")
                    nc.scalar.activation(sii[:], psi[:], AF.Sigmoid,
                                         bias=gbi_sb[:, i:i + 1])
                    si[i] = sii

                alpha = [None] * NCT
                for i in range(NCT):
                    al = ap_.tile([NP, L], F32, name=f"al_{t}_{i}", tag="alpha")
                    nc.scalar.activation(al[:], sf[i][:], AF.Exp,
                                         scale=msp_sb[:, i:i + 1])
                    alpha[i] = al
                a2 = [None] * NCT
                for i in range(NCT):
                    # a2 = alpha^2 <= 1 exactly, so 1+1e-6-a2 stays positive
                    a2i = a2p.tile([NP, L], F32, name=f"a2_{t}_{i}", tag="a2")
                    nc.gpsimd.tensor_tensor(a2i[:], alpha[i][:], alpha[i][:],
                                            OP.mult)
                    a2[i] = a2i
                beta = [None] * NCT
                for i in range(NCT):
                    be = bp.tile([NP, L], BF16, name=f"be_{t}_{i}", tag="beta")
                    nc.scalar.activation(be[:], a2[i][:], AF.Sqrt,
                                         scale=-1.0, bias=onepb[:, 0:1])
                    beta[i] = be

                h_cur = [None] * NCT
                ac_cur = [None] * NCT
                for i in range(NCT):
                    sz = szp.tile([NP, L], BF16, name=f"sz_{t}_{i}", tag="sz")
                    nc.vector.tensor_tensor(sz[:], si[i][:], zcb[i][:], OP.mult)
                    xs = xsp.tile([NP, L], BF16, name=f"xs_{t}_{i}", tag="xs")
                    nc.vector.tensor_tensor(xs[:], sz[:], beta[i][:], OP.mult)

                    h = hp.tile([NP, L], BF16, name=f"h_{t}_{i}", tag="h")
                    h_init = 0.0 if t == 0 else h_prev[i][:, L - 1:L]
                    nc.vector.tensor_tensor_scan(h[:], alpha[i][:], xs[:],
                                                 h_init, op0=OP.mult,
                                                 op1=OP.add)
                    h_cur[i] = h
                    st_rr[i % 3].dma_start(
                        h_d.ap()[i * NP:(i + 1) * NP, t * L:(t + 1) * L], h[:])
                    if t == n_tiles - 1:
                        nc.scalar.dma_start(
                            carry_loc.ap()[0:1, i * NP:(i + 1) * NP],
                            h[:, L - 1:L])

                    ac = acp.tile([NP, L], BF16, name=f"ac_{t}_{i}", tag="ac")
                    a_init = 1.0 if t == 0 else ac_prev[i][:, L - 1:L]
                    nc.vector.tensor_tensor_scan(ac[:], alpha[i][:],
                                                 ones_f[:, 0:L], a_init,
                                                 op0=OP.mult, op1=OP.mult)
                    ac_cur[i] = ac
                    st_rr[(i + 1) % 3].dma_start(
                        ac_d.ap()[i * NP:(i + 1) * NP, t * L:(t + 1) * L], ac[:])
                h_prev = h_cur
                ac_prev = ac_cur

                # gate half of the input proj (needed only in phase 2)
                for m in range(NCT):
                    ps = pmm.tile([NP, L], F32, name=f"g_ps_{t}_{m}", tag="mm")
                    for k in range(NCT):
                        lhs = wi_sb[:, k * H2 + m * NP: k * H2 + (m + 1) * NP]
                        nc.tensor.matmul(ps[:], lhs, xb[k][:],
                                         start=(k == 0), stop=(k == NCT - 1))
                    g_bf = gp.tile([NP, L], BF16, name=f"g_{t}_{m}", tag="g")
                    nc.vector.tensor_tensor(g_bf[:], ps[:], s[:], OP.mult)
                    st_rr[m % 3].dma_start(
                        g_d.ap()[m * NP:(m + 1) * NP, t * L:(t + 1) * L],
                        g_bf[:])

            # ---- pairwise carry exchange ----
            nc.gpsimd.collective_compute(
                "AllGather", OP.bypass,
                replica_groups=[[0, 1], [2, 3], [4, 5], [6, 7]],
                ins=[carry_loc.ap()], outs=[carry_gth.ap()])

        # =========================== PHASE 2 ===========================
        with ExitStack() as p2:
            wpool2 = p2.enter_context(tc.tile_pool(name="w2", bufs=1))
            wgr_sb = wpool2.tile([NP, NCT * HID], BF16, name="wgr_sb")
            wsh_sb = wpool2.tile([NP, 2 * NCT * D], BF16, name="wsh_sb")

            cg = cpool.tile([NP, NCT], BF16, name="cg")
            for i in range(NCT):
                nc.scalar.dma_start(
                    cg[:, i:i + 1],
                    carry_gth.ap()[0:1, i * NP:(i + 1) * NP].rearrange(
                        "a c -> c a"))
            carrym = cpool.tile([NP, NCT], F32, name="carrym")
            nc.vector.tensor_scalar(carrym[:], cg[:], cmask_sb[:, 0:1], None,
                                    op0=OP.mult)

            xrp = p2.enter_context(tc.tile_pool(name="xrp", bufs=9))
            hp2 = p2.enter_context(tc.tile_pool(name="hp2", bufs=6))
            hfp = p2.enter_context(tc.tile_pool(name="hfp", bufs=2))
            ggp = p2.enter_context(tc.tile_pool(name="ggp", bufs=3))
            ghp = p2.enter_context(tc.tile_pool(name="ghp", bufs=8))
            x1p = p2.enter_context(tc.tile_pool(name="x1p", bufs=9))
            sq2p = p2.enter_context(tc.tile_pool(name="sq2p", bufs=3))
            s2p = p2.enter_context(tc.tile_pool(name="s2p", bufs=3))
            t2p = p2.enter_context(tc.tile_pool(name="t2p", bufs=3))
            t2gp = p2.enter_context(tc.tile_pool(name="t2gp", bufs=3))
            gvp = p2.enter_context(tc.tile_pool(name="gvp", bufs=16))
            op_ = p2.enter_context(tc.tile_pool(name="op", bufs=2))
            pmm2 = p2.enter_context(
                tc.tile_pool(name="pmm2", bufs=3, space="PSUM"))
            pgro = p2.enter_context(
                tc.tile_pool(name="pgro", bufs=4, space="PSUM"))
            pssq2 = p2.enter_context(
                tc.tile_pool(name="pssq2", bufs=1, space="PSUM"))

            def load_tile2(t):
                hr = [None] * NCT
                acr = [None] * NCT
                gr = [None] * NCT
                xr = [None] * NCT
                for i in range(NCT):
                    hri = hp2.tile([NP, L2], BF16, name=f"hr_{t}_{i}",
                                   tag="hr")
                    nc.sync.dma_start(
                        hri[:],
                        h_d.ap()[i * NP:(i + 1) * NP, t * L2:(t + 1) * L2])
                    hr[i] = hri
                    aci = hp2.tile([NP, L2], BF16, name=f"acr_{t}_{i}",
                                   tag="acr")
                    nc.gpsimd.dma_start(
                        aci[:],
                        ac_d.ap()[i * NP:(i + 1) * NP, t * L2:(t + 1) * L2])
                    acr[i] = aci
                    gri = hp2.tile([NP, L2], BF16, name=f"gr_{t}_{i}",
                                   tag="gr")
                    nc.scalar.dma_start(
                        gri[:],
                        g_d.ap()[i * NP:(i + 1) * NP, t * L2:(t + 1) * L2])
                    gr[i] = gri
                    xri = xrp.tile([NP, L2], BF16, name=f"xr_{t}_{i}",
                                   tag="xr")
                    nc.sync.dma_start(
                        xri[:],
                        xb_d.ap()[i * NP:(i + 1) * NP, t * L2:(t + 1) * L2])
                    xr[i] = xri
                return hr, acr, gr, xr

            pre = {0: load_tile2(0)}
            for k in range(NCT):
                nc.sync.dma_start(wgr_sb[:, k * HID:(k + 1) * HID],
                                  wgr_d.ap()[k * NP:(k + 1) * NP, :])
            for k in range(2 * NCT):
                nc.gpsimd.dma_start(wsh_sb[:, k * D:(k + 1) * D],
                                    wsh_d.ap()[k * NP:(k + 1) * NP, :])
            if n_tiles2 > 1:
                pre[1] = load_tile2(1)

            for t in range(n_tiles2):
                hr, acr, gr, xr = pre.pop(t) if t in pre else load_tile2(t)
                gh = [None] * NCT
                for i in range(NCT):
                    ggi = ggp.tile([NP, L2], BF16, name=f"gg_{t}_{i}", tag="gg")
                    if gelu_approx:
                        sgi = ggp.tile([NP, L2], F32, name=f"sg_{t}_{i}",
                                       tag="sg")
                        nc.scalar.activation(sgi[:], gr[i][:], AF.Sigmoid,
                                             scale=1.702)
                        nc.vector.tensor_tensor(ggi[:], gr[i][:], sgi[:],
                                                OP.mult)
                    else:
                        nc.scalar.activation(ggi[:], gr[i][:], GELU)
                    hfi = hfp.tile([NP, L2], BF16, name=f"hf_{t}_{i}", tag="hf")
                    nc.vector.scalar_tensor_tensor(hfi[:], acr[i][:],
                                                   carrym[:, i:i + 1],
                                                   hr[i][:],
                                                   op0=OP.mult, op1=OP.add)
                    ghi = ghp.tile([NP, L2], BF16, name=f"gh_{t}_{i}", tag="gh")
                    nc.gpsimd.tensor_tensor(ghi[:], ggi[:], hfi[:], OP.mult)
                    gh[i] = ghi

                # output proj + residual -> x1 (bf16)
                x1 = [None] * NCT
                for m in range(NCT):
                    ps = pmm2.tile([NP, L2], F32, name=f"o_ps_{t}_{m}",
                                   tag="mm2")
                    for k in range(NCT):
                        lhs = wo_sb[:, k * D + m * NP: k * D + (m + 1) * NP]
                        nc.tensor.matmul(ps[:], lhs, gh[k][:],
                                         start=(k == 0), stop=(k == NCT - 1))
                    x1m = x1p.tile([NP, L2], BF16, name=f"x1_{t}_{m}", tag="x1")
                    nc.vector.tensor_tensor(x1m[:], ps[:], xr[m][:], OP.add)
                    x1[m] = x1m

                # x1 norm scale (broadcast), applied at drains
                ssq2 = pssq2.tile([NP, L2], F32, name=f"ssq2_{t}", tag="ssq2")
                for i in range(NCT):
                    xsq = sq2p.tile([NP, L2], BF16, name=f"x1sq_{t}_{i}",
                                    tag="x1sq")
                    nc.scalar.activation(xsq[:], x1[i][:], AF.Square)
                    nc.tensor.matmul(ssq2[:], ones_bf[:], xsq[:],
                                     start=(i == 0), stop=(i == NCT - 1))
                lssq2 = s2p.tile([NP, L2], F32, name=f"lssq2_{t}", tag="lssq2", bufs=2)
                nc.scalar.activation(lssq2[:], ssq2[:], AF.Ln,
                                     bias=epsb[:, 0:1])
                s2 = s2p.tile([NP, L2], F32, name=f"s2_{t}", tag="s2")
                nc.scalar.activation(s2[:], lssq2[:], AF.Exp, scale=-0.5)

                # grow proj: gate2 rows [0:2D), v rows [2D:4D)
                gv = [None] * (2 * NCT)
                for hm in range(2 * NCT):
                    psg = pgro.tile([NP, L2], F32, name=f"g2_ps_{t}_{hm}",
                                    tag="mm2g")
                    for k in range(NCT):
                        lhs = wgr_sb[:, k * HID + hm * NP:
                                     k * HID + (hm + 1) * NP]
                        nc.tensor.matmul(psg[:], lhs, x1[k][:],
                                         start=(k == 0), stop=(k == NCT - 1))
                    psv = pgro.tile([NP, L2], F32, name=f"v_ps_{t}_{hm}",
                                    tag="mm2g")
                    for k in range(NCT):
                        lhs = wgr_sb[:, k * HID + H2 + hm * NP:
                                     k * HID + H2 + (hm + 1) * NP]
                        nc.tensor.matmul(psv[:], lhs, x1[k][:],
                                         start=(k == 0), stop=(k == NCT - 1))
                    t2 = t2p.tile([NP, L2], BF16, name=f"t2_{t}_{hm}", tag="t2")
                    nc.vector.tensor_tensor(t2[:], psg[:], s2[:], OP.mult)
                    t2g = t2gp.tile([NP, L2], BF16, name=f"t2g_{t}_{hm}",
                                    tag="t2g")
                    if gelu_approx:
                        sg2 = t2gp.tile([NP, L2], F32, name=f"sg2_{t}_{hm}",
                                        tag="sg2")
                        nc.scalar.activation(sg2[:], t2[:], AF.Sigmoid,
                                             scale=1.702)
                        nc.vector.tensor_tensor(t2g[:], t2[:], sg2[:],
                                                OP.mult)
                    else:
                        nc.scalar.activation(t2g[:], t2[:], GELU)
                    gvi = gvp.tile([NP, L2], BF16, name=f"gv_{t}_{hm}",
                                   tag="gv")
                    nc.vector.tensor_tensor(gvi[:], t2g[:], psv[:], OP.mult)
                    gv[hm] = gvi

                # shrink proj (x s2) + residual -> out
                for m in range(NCT):
                    ps = pmm2.tile([NP, L2], F32, name=f"s_ps_{t}_{m}",
                                   tag="mm2")
                    for k in range(2 * NCT):
                        lhs = wsh_sb[:, k * D + m * NP: k * D + (m + 1) * NP]
                        nc.tensor.matmul(ps[:], lhs, gv[k][:],
                                         start=(k == 0), stop=(k == 2 * NCT - 1))
                    om1 = op_.tile([NP, L2], F32, name=f"om1_{t}_{m}",
                                   tag="om1")
                    nc.vector.tensor_tensor(om1[:], ps[:], s2[:], OP.mult)
                    om = op_.tile([NP, L2], F32, name=f"out_{t}_{m}", tag="out")
                    nc.vector.tensor_tensor(om[:], om1[:], x1[m][:], OP.add)
                    nc.scalar.dma_start(
                        out_d.ap()[m * NP:(m + 1) * NP, t * L2:(t + 1) * L2],
                        om[:])

    nc.compile()
    return nc


def host_prepare(inputs, T_core, n_cores=N_CORES):
    """Build per-core in_maps from full inputs."""
    x = np.asarray(inputs["x"], np.float32)            # [B, T, D]
    B, T, _ = x.shape
    halves = n_cores // B
    assert T == halves * T_core

    gam1 = np.asarray(inputs["hawk_norm_gamma"], np.float32)
    gam2 = np.asarray(inputs["gmlp_norm_gamma"], np.float32)
    scale1 = gam1 * np.sqrt(D)
    scale2 = gam2 * np.sqrt(D)

    wi = (np.asarray(inputs["input_w"], np.float32) * scale1[None, :]).T
    wg = np.asarray(inputs["gates_w"], np.float32).T
    wo = np.asarray(inputs["output_w"], np.float32).T
    wgr = (np.asarray(inputs["grow_w"], np.float32) * scale2[None, :]).T
    wsh = np.asarray(inputs["shrink_w"], np.float32).T

    fb = np.asarray(inputs["forget_base"], np.float64)
    msp = (-8.0 * np.log1p(np.exp(fb))).astype(np.float32)

    def chan_layout(v):  # [D] -> [128, 8] with [p, i] = v[128*i + p]
        return np.ascontiguousarray(v.reshape(NCT, NP).T)

    gb = np.asarray(inputs["gates_b"], np.float32)
    cw = np.asarray(inputs["conv_w"], np.float32)[:, 0, :]   # [D, K]
    cb = np.asarray(inputs["conv_b"], np.float32)

    shared = {
        "wi": wi.astype(_BF), "wg": wg.astype(_BF), "wo": wo.astype(_BF),
        "wgr": wgr.astype(_BF), "wsh": wsh.astype(_BF),
        "msp": chan_layout(msp), "msp2": chan_layout(2.0 * msp),
        "gbf": chan_layout(gb[:D]), "gbi": chan_layout(gb[D:]),
        "cw": np.concatenate([chan_layout(cw[:, k]) for k in range(KCONV)],
                             axis=1),
        "cb": chan_layout(cb),
    }
    in_maps = []
    for core in range(n_cores):
        b, h = core // halves, core % halves
        xf = np.zeros((D, 3 + T_core), np.float32)
        xf[:, 3:] = x[b, h * T_core:(h + 1) * T_core, :].T
        if h > 0:
            xf[:, 0:3] = x[b, h * T_core - 3:h * T_core, :].T
        m = dict(shared)
        m["x"] = xf
        m["cmask"] = np.full((NP, 1), 1.0 if h > 0 else 0.0, np.float32)
        in_maps.append(m)
    return in_maps


def assemble_output(results, B, T, T_core, n_cores=N_CORES):
    halves = n_cores // B
    out = np.empty((B, T, D), np.float32)
    for core in range(n_cores):
        b, h = core // halves, core % halves
        out[b, h * T_core:(h + 1) * T_core, :] = results[core]["out"].T
    return out


_PROG_CACHE = {}


def kernel(**inputs) -> np.ndarray:
    x = np.asarray(inputs["x"])
    B, T, _ = x.shape
    T_core = T * B // N_CORES
    L = 512 if T_core % 512 == 0 else T_core // 4
    key = (T_core, L)
    if key not in _PROG_CACHE:
        _PROG_CACHE[key] = build_program(T_core, L)
    nc = _PROG_CACHE[key]
    in_maps = host_prepare(inputs, T_core)
    res = run_bass_kernel_spmd(nc, in_maps, list(range(N_CORES)))
    return assemble_output(res.results, B, T, T_core)
